# revision 4
# baseline (speedup 1.0000x reference)
"""AdditiveAttention Trainium2 kernel (8 NeuronCores, data-parallel over batch).

Math: scores[b,q,k] = sum_h wv[h] * tanh(qp[b,q,h] + kp[b,k,h]) with
qp = queries @ Wq^T, kp = keys @ Wk^T, then length-masked softmax over k and
attn @ values.

Device strategy (per core, 2 batches):
  tanh(x) ~= sum_t c_t * sin(w_t x) with odd-harmonic ladder w_t = (2t-1)*w0.
  sin(w(a+b)) = sin(wa)cos(wb) + cos(wa)sin(wb) turns scoring into matmuls
  with contraction 2*T*H. sin/cos of all harmonics come from one in-domain
  ACT Sin pair via the Chebyshev-style recurrence f_{k+2} = 2cos(2w0 x) f_k
  - f_{k-2}. Softmax needs no max pass (|scores| <= sum|c||wv|_1 is small);
  masking multiplies V (and an appended ones-column that yields Z) by the
  0/1 mask so exp needs no masking; normalization happens after the AV
  matmul on [Q, DV] instead of [Q, K].
"""

import os
import sys

for _p in ("/opt/trn_rl_repo", os.path.expanduser("~/.axon_site/_ro/trn_rl_repo")):
    if os.path.isdir(_p) and _p not in sys.path:
        sys.path.insert(0, _p)

import math

import ml_dtypes
import numpy as np

import concourse.bass as bass
import concourse.mybir as mybir
import concourse.tile as tile
from concourse import bacc
from concourse.bass_utils import run_bass_kernel_spmd

BF16 = ml_dtypes.bfloat16
F32 = mybir.dt.float32
BF = mybir.dt.bfloat16

B, Q, K, H = 16, 512, 512, 64
DQ = DK = DV = 256
P = 128
NCORES = 8
SLOTS = 2

# Odd-harmonic sine expansion of tanh, fit under the N(0, 2) weight of
# qp+kp (both projections are ~N(0,1) for the given input scaling).
W0 = 0.4010
CS = np.array([1.193248, 0.247628, 0.069403, 0.018763, 0.006754], np.float64)
T = len(CS)

AF = mybir.ActivationFunctionType
ALU = mybir.AluOpType

_COMPILE_CACHE = {}

# test.py hooks: set trace/profile behaviour and capture raw results.
TRACE = False
LAST_RESULTS = None


def _build(kt_bounds):
    """Build the SPMD graph. kt_bounds[s] = number of 128-wide k tiles that
    slot s must process (max over cores of ceil(valid_len/128))."""
    nc = bacc.Bacc()

    qT = nc.declare_dram_parameter("qT", [SLOTS, DQ, Q], BF, isOutput=False)
    kT = nc.declare_dram_parameter("kT", [SLOTS, DK, K], BF, isOutput=False)
    vals = nc.declare_dram_parameter("vals", [SLOTS, K, DV], BF, isOutput=False)
    msk = nc.declare_dram_parameter("msk", [SLOTS, K], F32, isOutput=False)
    wqT = nc.declare_dram_parameter("wqT", [DQ, H], BF, isOutput=False)
    wkT = nc.declare_dram_parameter("wkT", [DK, H], BF, isOutput=False)
    cwv = nc.declare_dram_parameter("cwv", [P, T], F32, isOutput=False)
    out = nc.declare_dram_parameter("out", [SLOTS, Q, DV], F32, isOutput=True)

    with tile.TileContext(nc) as tc:
        with (
            tc.tile_pool(name="singles", bufs=1) as singles,
            tc.tile_pool(name="io", bufs=2) as io,
            tc.tile_pool(name="lad", bufs=2) as lad,
            tc.tile_pool(name="feat", bufs=2) as feat,
            tc.tile_pool(name="esb", bufs=2) as esb,
            tc.tile_pool(name="osb", bufs=4) as osb,
            tc.tile_pool(name="pproj", bufs=2, space="PSUM") as pproj,
            tc.tile_pool(name="psc", bufs=2, space="PSUM") as psc,
            tc.tile_pool(name="pav", bufs=2, space="PSUM") as pav,
        ):
            wq_sb = singles.tile([P, 2, H], BF)
            nc.sync.dma_start(wq_sb[:], wqT.rearrange("(c p) h -> p c h", p=P))
            wk_sb = singles.tile([P, 2, H], BF)
            nc.sync.dma_start(wk_sb[:], wkT.rearrange("(c p) h -> p c h", p=P))
            cwv_sb = singles.tile([P, T], F32)
            nc.sync.dma_start(cwv_sb[:], cwv[:, :])

            # -------- phase A: projections, harmonic ladder, features ------
            fA = [[None] * T for _ in range(SLOTS)]
            fB = [[None] * T for _ in range(SLOTS)]
            vaug = [[None] * 4 for _ in range(SLOTS)]
            for s in range(SLOTS):
                ktn = kt_bounds[s]

                q_sb = io.tile([P, 2, Q], BF, tag="q_sb")
                nc.sync.dma_start(q_sb[:], qT[s].rearrange("(c p) q -> p c q", p=P))
                k_sb = io.tile([P, 2, K], BF, tag="k_sb")
                nc.sync.dma_start(k_sb[:], kT[s].rearrange("(c p) k -> p c k", p=P))

                mask_sb = io.tile([P, 4], F32, tag="mask_sb")
                nc.sync.dma_start(
                    mask_sb[:, 0:ktn],
                    msk[s].rearrange("(kt p) -> p kt", p=P)[:, 0:ktn],
                )
                for kt in range(ktn):
                    va = io.tile([P, DV + 1], BF, tag=f"vaug{kt}")
                    vaug[s][kt] = va
                    nc.sync.dma_start(
                        va[:, 0:DV], vals[s, kt * P : (kt + 1) * P, :]
                    )
                    nc.vector.memset(va[:, DV : DV + 1], 1.0)
                    nc.vector.tensor_scalar_mul(va[:], va[:], mask_sb[:, kt : kt + 1])

                # qp/kp projections -> one stacked psum tile qk = [qp; kp]
                # (kp chain targets partitions 64:128 via tile_position
                # col-group; the two chains can run concurrently on the PE)
                qk = pproj.tile([P, Q], F32, tag="qk")
                for c in range(2):
                    nc.tensor.matmul(
                        qk[0:H, :],
                        wq_sb[:, c, :],
                        q_sb[:, c, :],
                        start=(c == 0),
                        stop=(c == 1),
                        tile_position=(0, 0),
                    )
                for c in range(2):
                    nc.tensor.matmul(
                        qk[H:P, :],
                        wk_sb[:, c, :],
                        k_sb[:, c, :],
                        start=(c == 0),
                        stop=(c == 1),
                        tile_position=(0, H),
                    )

                # base harmonics via ACT (args stay within Sin's |x|<pi domain)
                s1 = lad.tile([P, Q], BF, tag="s1")
                nc.scalar.activation(s1[:], qk[:], AF.Sin, scale=W0)
                sh = lad.tile([P, Q], BF, tag="sh")
                nc.scalar.activation(sh[:], qk[:], AF.Sin, scale=W0 / 2.0)
                sq1 = lad.tile([P, Q], BF, tag="sq1")
                nc.scalar.activation(sq1[:], s1[:], AF.Square)
                sqh = lad.tile([P, Q], BF, tag="sqh")
                nc.scalar.activation(sqh[:], sh[:], AF.Square)

                c2x2 = lad.tile([P, Q], BF, tag="c2x2")  # 2*cos(2*w0*x)
                nc.vector.tensor_scalar(c2x2[:], sq1[:], -4.0, 2.0, ALU.mult, ALU.add)
                c1 = lad.tile([P, Q], BF, tag="c1")  # cos(w0*x)
                nc.vector.tensor_scalar(c1[:], sqh[:], -2.0, 1.0, ALU.mult, ALU.add)

                # odd-harmonic Chebyshev ladders:
                #   s_{k+2} = 2c2*s_k - s_{k-2}  (s_{-1} = -s1)
                #   c_{k+2} = 2c2*c_k - c_{k-2}  (c_{-1} =  c1)
                # sin ladder on DVE, cos ladder on GPSIMD (parallel engines).
                S = [s1]
                Cc = [c1]
                for t in range(1, T):
                    st = lad.tile([P, Q], BF, tag=f"s{t}")
                    tmp = lad.tile([P, Q], BF, tag="ltmp")
                    nc.vector.tensor_tensor(tmp[:], c2x2[:], S[-1][:], ALU.mult)
                    if t == 1:
                        nc.vector.tensor_tensor(st[:], tmp[:], s1[:], ALU.add)
                    else:
                        nc.vector.tensor_tensor(st[:], tmp[:], S[-2][:], ALU.subtract)
                    S.append(st)

                    ct = lad.tile([P, Q], BF, tag=f"c{t}")
                    tmpc = lad.tile([P, Q], BF, tag="ltmpc")
                    nc.gpsimd.tensor_tensor(tmpc[:], c2x2[:], Cc[-1][:], ALU.mult)
                    prev = Cc[0] if t == 1 else Cc[-2]
                    nc.gpsimd.tensor_tensor(ct[:], tmpc[:], prev[:], ALU.subtract)
                    Cc.append(ct)

                # assemble matmul chunks:
                #   fA_t = [sinA_t ; cosA_t], fB_t = [cosB_t ; sinB_t] * cwv_t
                # (A parts live in partitions 0:64 of the qk-stacked tiles,
                #  B parts in 64:128; DMA does the partition moves)
                for t in range(T):
                    fa = feat.tile([P, Q], BF, tag=f"fA{t}")
                    nc.sync.dma_start(fa[0:H, :], S[t][0:H, :])
                    nc.sync.dma_start(fa[H:P, :], Cc[t][0:H, :])
                    fb = feat.tile([P, K], BF, tag=f"fB{t}")
                    nc.sync.dma_start(fb[0:H, :], Cc[t][H:P, :])
                    nc.sync.dma_start(fb[H:P, :], S[t][H:P, :])
                    nc.vector.tensor_scalar_mul(fb[:], fb[:], cwv_sb[:, t : t + 1])
                    fA[s][t] = fa
                    fB[s][t] = fb

            # -------- phase B: scores, softmax, AV, output -----------------
            for s in range(SLOTS):
                ktn = kt_bounds[s]
                e_tiles = []
                for kt in range(ktn):
                    sc = psc.tile([P, Q], F32, tag="sc")
                    for t in range(T):
                        nc.tensor.matmul(
                            sc[:],
                            fB[s][t][:, kt * P : (kt + 1) * P],
                            fA[s][t][:],
                            start=(t == 0),
                            stop=(t == T - 1),
                        )
                    e_kt = esb.tile([P, Q], BF, tag=f"e{kt}")
                    nc.scalar.activation(e_kt[:], sc[:], AF.Exp)
                    e_tiles.append(e_kt)

                for qt in range(Q // P):
                    o_ps = pav.tile([P, DV + 1], F32, tag="o_ps")
                    for kt in range(ktn):
                        nc.tensor.matmul(
                            o_ps[:],
                            e_tiles[kt][:, qt * P : (qt + 1) * P],
                            vaug[s][kt][:],
                            start=(kt == 0),
                            stop=(kt == ktn - 1),
                        )
                    rz = osb.tile([P, 1], F32, tag="rz")
                    nc.vector.reciprocal(rz[:], o_ps[:, DV : DV + 1])
                    o_sb = osb.tile([P, DV], F32, tag="o_sb")
                    nc.vector.tensor_scalar_mul(o_sb[:], o_ps[:, 0:DV], rz[:])
                    nc.sync.dma_start(out[s, qt * P : (qt + 1) * P, :], o_sb[:])

    nc.finalize()
    return nc


def kernel(queries, keys, values, valid_lens, Wq, Wk, wv):
    global LAST_RESULTS
    queries = np.asarray(queries, np.float32)
    keys = np.asarray(keys, np.float32)
    values = np.asarray(values, np.float32)
    vl = np.asarray(valid_lens).astype(np.int64)
    Wq = np.asarray(Wq, np.float32)
    Wk = np.asarray(Wk, np.float32)
    wv = np.asarray(wv, np.float32)

    # slot 0 takes the 8 longest sequences, slot 1 the rest; per-slot k-tile
    # bounds are the max over cores so one SPMD program fits all cores.
    order = np.argsort(-vl, kind="stable")
    slot_b = [order[:NCORES], order[NCORES:]]
    kt_bounds = tuple(
        max(1, math.ceil(int(vl[sb].max()) / P)) for sb in slot_b
    )

    key = kt_bounds
    if key not in _COMPILE_CACHE:
        _COMPILE_CACHE[key] = _build(kt_bounds)
    nc = _COMPILE_CACHE[key]

    qTb = np.ascontiguousarray(queries.transpose(0, 2, 1)).astype(BF16)
    kTb = np.ascontiguousarray(keys.transpose(0, 2, 1)).astype(BF16)
    valsb = values.astype(BF16)
    maskb = (np.arange(K)[None, :] < vl[:, None]).astype(np.float32)
    wqTb = np.ascontiguousarray(Wq.T).astype(BF16)
    wkTb = np.ascontiguousarray(Wk.T).astype(BF16)
    cwv_h = (CS[None, :] * wv[:, None].astype(np.float64)).astype(np.float32)  # [H, T]
    cwv_full = np.concatenate([cwv_h, cwv_h], axis=0)  # [128, T]

    in_maps = []
    for i in range(NCORES):
        bsel = np.array([slot_b[0][i], slot_b[1][i]])
        in_maps.append(
            {
                "qT": qTb[bsel],
                "kT": kTb[bsel],
                "vals": valsb[bsel],
                "msk": maskb[bsel],
                "wqT": wqTb,
                "wkT": wkTb,
                "cwv": cwv_full,
            }
        )

    res = run_bass_kernel_spmd(
        nc, in_maps, core_ids=list(range(NCORES)), trace=TRACE
    )
    LAST_RESULTS = res

    out = np.empty((B, Q, DV), np.float32)
    for i in range(NCORES):
        o = np.asarray(res.results[i]["out"])
        out[slot_b[0][i]] = o[0]
        out[slot_b[1][i]] = o[1]
    return out
